# revision 13
# baseline (speedup 1.0000x reference)
# Trainium2 Bass kernel for nn_CompressedGPT2Attention.
#
# Model: B=2, S=2048, D=1024, H=16 heads of HD=64.
#   qkv = x @ c_attn_w + c_attn_b ; causal attention per head;
#   per-head symmetric projector on the attention output; out = attn @ c_proj_w + b.
#
# Sharding (megatron-style tensor parallel over heads, 8 cores x 2 heads):
#   - every core gets the full hidden_states
#   - c_attn (q,k,v) columns + projectors + c_proj rows are sharded by head
#   - each core computes a full-shape partial of the c_proj output; the
#     all-reduce after c_proj is done on the host (partials are summed there).
#
# On-core layout strategy: activations are kept feature-major ("transposed",
# features on SBUF partitions) so every matmul contracts over the partition
# dim without ever transposing big intermediates:
#   xT[d, s]   provided by the host (input marshalling) and cast to bf16
#              by the SWDGE load DMA
#   qT,kT[f,s] = W^T @ xT
#   scoresT[kj, qi] = kT^T-slice matmuls (two heads packed on the PE via
#                     tile_position row-tiling, K=64 each)
#   expT = exp(scoresT/8) on ScalarE, causal mask via gpsimd affine_select
#   v[s, hd]   computed directly in sequence-major layout (xT stationary)
#   attn_unT[hd, qi] accumulated over kj with lhsT = v; softmax sums
#                     ride along as a concurrent ones-column matmul
#   attnP_T[e, qi]  = projector matmul (two heads packed, K=64)
#   normalization   = (1/sums) broadcast across partitions with a K=1 matmul,
#                     then one VectorE multiply
#   outT[dout, s]   = c_proj partial, written back fp32; host sums over cores.

import numpy as np

B, S, D, H, HD = 2, 2048, 1024, 16, 64
BS = B * S
N_CORES = 8
HPC = H // N_CORES  # heads per core = 2

_CACHE = {}


def _build(nc):
    import concourse.bass as bass
    import concourse.mybir as mybir
    import concourse.tile as tile
    from contextlib import ExitStack

    f32 = mybir.dt.float32
    bf16 = mybir.dt.bfloat16
    AF = mybir.ActivationFunctionType
    OP = mybir.AluOpType

    x_d = nc.dram_tensor("xT", [D, BS], f32, kind="ExternalInput").ap()
    wqk_d = nc.dram_tensor("w_qk", [D, 2 * HPC * HD], f32, kind="ExternalInput").ap()
    wv_d = nc.dram_tensor("w_v", [D, HPC * HD], f32, kind="ExternalInput").ap()
    bqk_d = nc.dram_tensor("b_qk", [2 * HPC * HD], f32, kind="ExternalInput").ap()
    bv_d = nc.dram_tensor("b_v", [HPC * HD], f32, kind="ExternalInput").ap()
    wpr_d = nc.dram_tensor("w_pr", [HPC * HD, HD], f32, kind="ExternalInput").ap()
    wcp_d = nc.dram_tensor("w_cp", [HPC * HD, D], f32, kind="ExternalInput").ap()
    bcp_d = nc.dram_tensor("b_cp", [D], f32, kind="ExternalInput").ap()
    out_d = nc.dram_tensor("outT", [8, 128, BS], f32, kind="ExternalOutput").ap()

    F = HPC * HD  # 128 features per block (2 heads stacked)
    NB = BS // 512  # 8 s-blocks of 512
    KT = D // 128  # 8 contraction tiles

    with TileCtx(tile, nc) as tc:
        # ---------------- persistent tiles ----------------
        # tc.tile singles must be released in LIFO order, and their free
        # closures must be kept alive (GC of a discarded closure releases
        # the pool at a random trace point). xT goes last so it can be
        # freed right after the qkv phase.
        frees = []

        def ptile(shape, dtype, name):
            t, free = tc.tile(shape, dtype, name=name)
            frees.append(free)
            return t

        qT = ptile([128, BS], bf16, "qT")
        kTt = ptile([128, BS], bf16, "kTt")
        v_s = ptile([128, BS // 128, 128], bf16, "v_s")
        wqk_sb = ptile([128, KT, 2 * F], bf16, "wqk_sb")
        wv_sb = ptile([128, KT, F], bf16, "wv_sb")
        wpr_sb = ptile([128, HD], bf16, "wpr_sb")
        wcp_sb = ptile([128, D], bf16, "wcp_sb")
        bqk_sb = ptile([128, 2], f32, "bqk_sb")
        bv16 = ptile([128, 1], bf16, "bv16")
        bcp_sb = ptile([128, 8], f32, "bcp_sb")
        bias_sb = ptile([128, 8], f32, "bias_sb")
        pb_sb = ptile([128, 1], bf16, "pb_sb")
        ones16 = ptile([128, 1], bf16, "ones16")
        onesf = ptile([33, 64], f32, "onesf")
        # one tile per 512-wide s-block so c_proj can start per-block
        cpr = [ptile([128, 512], bf16, f"cpr{i}") for i in range(NB)]
        xT, xT_free = tc.tile([128, KT, BS], bf16, name="xT")

        # ---------------- constants + weights ----------------
        nc.any.memset(ones16[:], 1.0)
        nc.any.memset(onesf[:], 1.0)
        nc.gpsimd.dma_start(wqk_sb[:], wqk_d.rearrange("(kt p) f -> p kt f", p=128))
        nc.gpsimd.dma_start(wv_sb[:], wv_d.rearrange("(kt p) f -> p kt f", p=128))
        nc.gpsimd.dma_start(wpr_sb[:], wpr_d)
        nc.gpsimd.dma_start(wcp_sb[:], wcp_d)
        nc.sync.dma_start(bqk_sb[:], bqk_d.rearrange("(t p) -> p t", p=128))
        nc.gpsimd.dma_start(bv16[:], bv_d.rearrange("(t p) -> p t", p=128))
        nc.sync.dma_start(bcp_sb[:], bcp_d.rearrange("(t p) -> p t", p=128))

        # ---------------- load x^T (cast fp32 -> bf16 in-flight) ----------------
        with ExitStack() as phase1:
            for kt in range(KT):
                nc.gpsimd.dma_start(
                    xT[:, kt, :], x_d[kt * 128 : (kt + 1) * 128, :]
                )

            # ---------------- folded output bias ----------------
            # v's input bias commutes through softmax (rows sum to 1) and the
            # projectors:  bias_total[dout] = b_cp[dout] + (proj^T b_v) @ W_cp.
            qkv_ps = phase1.enter_context(
                tc.tile_pool(name="qkv_ps", bufs=3, space="PSUM")
            )
            aux1_ps = phase1.enter_context(
                tc.tile_pool(name="aux1_ps", bufs=2, space="PSUM")
            )
            ps_pb = aux1_ps.tile([128, 1], f32, tag="aux1")
            nc.tensor.matmul(
                ps_pb[0:64, :], wpr_sb[0:64, :], bv16[0:64, :],
                start=True, stop=True, tile_position=(0, 0),
                skip_group_check=True,
            )
            nc.tensor.matmul(
                ps_pb[64:128, :], wpr_sb[64:128, :], bv16[64:128, :],
                start=True, stop=True, tile_position=(64, 64),
                skip_group_check=True,
            )
            nc.scalar.activation(pb_sb[:], ps_pb[:], AF.Copy)
            for dt in range(8):
                ps_bt = aux1_ps.tile([128, 1], f32, tag="aux1")
                nc.tensor.matmul(
                    ps_bt[:], wcp_sb[:, dt * 128 : (dt + 1) * 128], pb_sb[:],
                    start=True, stop=True,
                )
                nc.vector.tensor_tensor(
                    bias_sb[:, dt : dt + 1], ps_bt[:], bcp_sb[:, dt : dt + 1],
                    OP.add,
                )

            # ---------------- q^T / k^T matmuls ----------------
            for ft in range(2):  # 0=q, 1=k
                dest = (qT, kTt)[ft]
                for sb in range(NB):
                    ps = qkv_ps.tile([128, 512], f32, tag="qkv")
                    for kt in range(KT):
                        nc.tensor.matmul(
                            ps[:],
                            wqk_sb[:, kt, ft * F : (ft + 1) * F],
                            xT[:, kt, sb * 512 : (sb + 1) * 512],
                            start=(kt == 0),
                            stop=(kt == KT - 1),
                        )
                    nc.scalar.activation(
                        dest[:, sb * 512 : (sb + 1) * 512], ps[:], AF.Identity,
                        bias=bqk_sb[:, ft : ft + 1],
                    )

            # ---------------- v, directly in sequence-major [s, hd] ----------
            # (xT tile is the stationary operand; no transposes needed; the
            #  v bias is folded into bias_sb above)
            for st in range(BS // 128):
                ps_v = qkv_ps.tile([128, 128], f32, tag="v")
                for kt in range(KT):
                    nc.tensor.matmul(
                        ps_v[:],
                        xT[:, kt, st * 128 : (st + 1) * 128],
                        wv_sb[:, kt, :],
                        start=(kt == 0),
                        stop=(kt == KT - 1),
                    )
                if st % 2 == 0:
                    nc.scalar.activation(v_s[:, st, :], ps_v[:], AF.Copy)
                else:
                    nc.vector.tensor_copy(v_s[:, st, :], ps_v[:])
        xT_free()

        # ---------------- attention ----------------
        with ExitStack() as phase2:
            sc_ps = phase2.enter_context(tc.tile_pool(name="sc_ps", bufs=4, space="PSUM"))
            attn_ps = phase2.enter_context(tc.tile_pool(name="attn_ps", bufs=2, space="PSUM"))
            aux_ps = phase2.enter_context(tc.tile_pool(name="aux_ps", bufs=2, space="PSUM"))
            epool = phase2.enter_context(tc.tile_pool(name="epool", bufs=6))
            spool = phase2.enter_context(tc.tile_pool(name="spool", bufs=2))

            for b in range(B):
                for qt in range(4):
                    blk = b * 4 + qt
                    qi = b * S + qt * 512
                    nkj = 4 * (qt + 1)
                    ps_attn = attn_ps.tile([128, 512], f32, tag="attn")
                    ps_sums = aux_ps.tile([33, 512], f32, tag="aux")
                    for kj in range(nkj):
                        kjc = b * S + kj * 128
                        psA = sc_ps.tile([128, 512], f32, tag="sc")
                        psB = sc_ps.tile([128, 512], f32, tag="sc")
                        nc.tensor.matmul(
                            psA[:], kTt[0:64, kjc : kjc + 128],
                            qT[0:64, qi : qi + 512],
                            start=True, stop=True, tile_position=(0, 0),
                        )
                        nc.tensor.matmul(
                            psB[:], kTt[64:128, kjc : kjc + 128],
                            qT[64:128, qi : qi + 512],
                            start=True, stop=True, tile_position=(64, 0),
                        )
                        eA = epool.tile([128, 512], bf16, tag="e")
                        eB = epool.tile([128, 512], bf16, tag="e")
                        nc.scalar.activation(eA[:], psA[:], AF.Exp, scale=0.125)
                        nc.scalar.activation(eB[:], psB[:], AF.Exp, scale=0.125)
                        p = kj - 4 * qt
                        if p >= 0:  # diagonal tile: causal mask
                            for e in (eA, eB):
                                nc.gpsimd.affine_select(
                                    e[:], e[:], pattern=[[1, 512]],
                                    base=-128 * p, channel_multiplier=-1,
                                    compare_op=OP.is_ge, fill=0.0,
                                )
                        first, last = kj == 0, kj == nkj - 1
                        vs = v_s[:, b * 16 + kj, :]
                        nc.tensor.matmul(
                            ps_attn[0:64, :], vs[:, 0:64], eA[:],
                            start=first, stop=last, tile_position=(0, 0),
                            skip_group_check=True,
                        )
                        nc.tensor.matmul(
                            ps_attn[64:128, :], vs[:, 64:128], eB[:],
                            start=first, stop=last, tile_position=(0, 64),
                            skip_group_check=True,
                        )
                        nc.tensor.matmul(
                            ps_sums[0:1, :], ones16[:, 0:1], eA[:],
                            start=first, stop=last, tile_position=(0, 0),
                            skip_group_check=True,
                        )
                        nc.tensor.matmul(
                            ps_sums[32:33, :], ones16[:, 0:1], eB[:],
                            start=first, stop=last, tile_position=(0, 32),
                            skip_group_check=True,
                        )

                    attn_sb = spool.tile([128, 512], bf16, tag="attn_sb")
                    nc.scalar.activation(attn_sb[:], ps_attn[:], AF.Copy)
                    rec = spool.tile([33, 512], f32, tag="rec")
                    nc.vector.reciprocal(rec[0:1, :], ps_sums[0:1, :])
                    nc.vector.reciprocal(rec[32:33, :], ps_sums[32:33, :])

                    ps_attnP = attn_ps.tile([128, 512], f32, tag="attn")
                    nc.tensor.matmul(
                        ps_attnP[0:64, :], wpr_sb[0:64, :], attn_sb[0:64, :],
                        start=True, stop=True, tile_position=(0, 0),
                        skip_group_check=True,
                    )
                    nc.tensor.matmul(
                        ps_attnP[64:128, :], wpr_sb[64:128, :], attn_sb[64:128, :],
                        start=True, stop=True, tile_position=(64, 64),
                        skip_group_check=True,
                    )
                    ps_bc = aux_ps.tile([128, 512], f32, tag="aux")
                    nc.tensor.matmul(
                        ps_bc[0:64, :], onesf[0:1, 0:64], rec[0:1, :],
                        start=True, stop=True, tile_position=(0, 0),
                        skip_group_check=True,
                    )
                    nc.tensor.matmul(
                        ps_bc[64:128, :], onesf[32:33, 0:64], rec[32:33, :],
                        start=True, stop=True, tile_position=(32, 64),
                        skip_group_check=True,
                    )
                    rec_bc = spool.tile([128, 512], f32, tag="rec_bc")
                    nc.scalar.activation(rec_bc[:], ps_bc[:], AF.Copy)
                    nc.vector.tensor_tensor(
                        cpr[blk][:], ps_attnP[:], rec_bc[:], OP.mult
                    )

        # ---------------- c_proj partial ----------------
        with ExitStack() as phase3:
            cp_ps = phase3.enter_context(tc.tile_pool(name="cp_ps", bufs=4, space="PSUM"))
            opool = phase3.enter_context(tc.tile_pool(name="opool", bufs=2))
            for dt in range(8):
                out_sb = opool.tile([128, BS], f32, tag="out_sb")
                for sb in range(NB):
                    ps = cp_ps.tile([128, 512], f32, tag="cp")
                    nc.tensor.matmul(
                        ps[:], wcp_sb[:, dt * 128 : (dt + 1) * 128], cpr[sb][:],
                        start=True, stop=True,
                    )
                    dst = out_sb[:, sb * 512 : (sb + 1) * 512]
                    if sb % 2 == 0:
                        nc.scalar.activation(
                            dst, ps[:], AF.Identity, bias=bias_sb[:, dt : dt + 1]
                        )
                    else:
                        nc.vector.tensor_scalar(
                            dst, ps[:], bias_sb[:, dt : dt + 1], None, OP.add
                        )
                nc.sync.dma_start(out_d[dt], out_sb[:])

        for free in reversed(frees):
            free()


class TileCtx:
    """Thin helper so _build can use `tc.tile` / `tc.tile_pool` uniformly."""

    def __init__(self, tile_mod, nc):
        self._tc = tile_mod.TileContext(nc)

    def __enter__(self):
        self._tc.__enter__()
        return self._tc

    def __exit__(self, *exc):
        return self._tc.__exit__(*exc)


def _shard_inputs(inputs):
    xT = np.ascontiguousarray(
        np.asarray(inputs["hidden_states"], dtype=np.float32).reshape(BS, D).T
    )
    Wa = np.asarray(inputs["c_attn_w"], dtype=np.float32)
    ba = np.asarray(inputs["c_attn_b"], dtype=np.float32)
    Wp = np.asarray(inputs["c_proj_w"], dtype=np.float32)
    bp = np.asarray(inputs["c_proj_b"], dtype=np.float32)
    proj = np.asarray(inputs["projectors"], dtype=np.float32)

    in_maps = []
    F = HPC * HD
    for c in range(N_CORES):
        sl = slice(c * F, (c + 1) * F)
        in_maps.append(
            {
                "xT": xT,
                "w_qk": np.ascontiguousarray(
                    np.concatenate([Wa[:, sl], Wa[:, D + c * F : D + (c + 1) * F]], axis=1)
                ),
                "w_v": np.ascontiguousarray(Wa[:, 2 * D + c * F : 2 * D + (c + 1) * F]),
                "b_qk": np.ascontiguousarray(
                    np.concatenate([ba[sl], ba[D + c * F : D + (c + 1) * F]])
                ),
                "b_v": np.ascontiguousarray(ba[2 * D + c * F : 2 * D + (c + 1) * F]),
                "w_pr": np.ascontiguousarray(
                    proj[HPC * c : HPC * (c + 1)].reshape(F, HD)
                ),
                "w_cp": np.ascontiguousarray(Wp[sl, :]),
                "b_cp": bp if c == 0 else np.zeros_like(bp),
            }
        )
    return in_maps


def _get_nc():
    if "nc" not in _CACHE:
        from concourse import bacc

        nc = bacc.Bacc("TRN2", debug=False, num_devices=N_CORES)
        _build(nc)
        # Bacc.compile() runs generate_event_semaphores, which spills
        # per-instruction sync waits beyond the single HW wait slot into
        # separate EventSemaphore instructions — without it walrus fails
        # with "Too many sync wait commands".
        nc.compile()
        _CACHE["nc"] = nc
    return _CACHE["nc"]


def _run(inputs, trace=False, trace_kwargs=None):
    from concourse.bass_utils import run_bass_kernel_spmd

    nc = _get_nc()
    in_maps = _shard_inputs(inputs)
    res = run_bass_kernel_spmd(
        nc,
        in_maps,
        core_ids=list(range(N_CORES)),
        trace=trace,
        **(trace_kwargs or {}),
    )
    acc = np.zeros((8, 128, BS), dtype=np.float32)
    for r in res.results:
        acc += np.asarray(r["outT"], dtype=np.float32)
    out = acc.transpose(2, 0, 1).reshape(BS, D).reshape(B, S, D)
    return np.ascontiguousarray(out), res


def kernel(**inputs) -> np.ndarray:
    out, _ = _run(inputs, trace=False)
    return out


def simulate_core(inputs, core=0):
    """CoreSim one core's program (for correctness debugging). Returns outT."""
    from concourse.bass_interp import CoreSim

    nc = _get_nc()
    in_maps = _shard_inputs(inputs)
    sim = CoreSim(nc, trace=False)
    for name, arr in in_maps[core].items():
        sim.tensor(name)[:] = arr
    sim.simulate()
    return np.array(sim.tensor("outT"))


# revision 20
# speedup vs baseline: 1.2318x; 1.2318x over previous
# Trainium2 Bass kernel for nn_CompressedGPT2Attention.
#
# Model: B=2, S=2048, D=1024, H=16 heads of HD=64.
#   qkv = x @ c_attn_w + c_attn_b ; causal attention per head;
#   per-head symmetric projector on the attention output; out = attn @ c_proj_w + b.
#
# Sharding (megatron-style tensor parallel over heads, 8 cores x 2 heads):
#   - every core gets the full hidden_states
#   - c_attn (q,k,v) columns + projectors + c_proj rows are sharded by head
#   - each core computes a full-shape partial of the c_proj output; the
#     all-reduce after c_proj is done on the host (partials are summed there).
#
# On-core layout strategy: activations are kept feature-major ("transposed",
# features on SBUF partitions) so every matmul contracts over the partition
# dim without ever transposing big intermediates:
#   xT[d, s]   provided by the host (input marshalling) and cast to bf16
#              by the SWDGE load DMA
#   qT,kT[f,s] = W^T @ xT
#   scoresT[kj, qi] = kT^T-slice matmuls (two heads packed on the PE via
#                     tile_position row-tiling, K=64 each)
#   expT = exp(scoresT/8) on ScalarE, causal mask via gpsimd affine_select
#   v[s, hd]   computed directly in sequence-major layout (xT stationary)
#   attn_unT[hd, qi] accumulated over kj with lhsT = v; softmax sums
#                     ride along as a concurrent ones-column matmul
#   attnP_T[e, qi]  = projector matmul (two heads packed, K=64)
#   normalization   = (1/sums) broadcast across partitions with a K=1 matmul,
#                     then one VectorE multiply
#   outT[dout, s]   = c_proj partial, written back fp32; host sums over cores.

import numpy as np

B, S, D, H, HD = 2, 2048, 1024, 16, 64
BS = B * S
N_CORES = 8
HPC = H // N_CORES  # heads per core = 2

_CACHE = {}
USE_CRIT = False  # tile_critical around paired matmuls faults the device


def _build(nc):
    import concourse.bass as bass
    import concourse.mybir as mybir
    import concourse.tile as tile
    from contextlib import ExitStack

    f32 = mybir.dt.float32
    bf16 = mybir.dt.bfloat16
    AF = mybir.ActivationFunctionType
    OP = mybir.AluOpType

    x_d = nc.dram_tensor("xT", [D, BS], f32, kind="ExternalInput").ap()
    wqk_d = nc.dram_tensor("w_qk", [D, 2 * HPC * HD], f32, kind="ExternalInput").ap()
    wv_d = nc.dram_tensor("w_v", [D, HPC * HD], f32, kind="ExternalInput").ap()
    bqk_d = nc.dram_tensor("b_qk", [2 * HPC * HD], f32, kind="ExternalInput").ap()
    bv_d = nc.dram_tensor("b_v", [HPC * HD], f32, kind="ExternalInput").ap()
    wpr_d = nc.dram_tensor("w_pr", [HPC * HD, HD], f32, kind="ExternalInput").ap()
    wcp_d = nc.dram_tensor("w_cp", [HPC * HD, D], f32, kind="ExternalInput").ap()
    bcp_d = nc.dram_tensor("b_cp", [D], f32, kind="ExternalInput").ap()
    out_d = nc.dram_tensor("outT", [8, 128, BS], f32, kind="ExternalOutput").ap()

    F = HPC * HD  # 128 features per block (2 heads stacked)
    NB = BS // 512  # 8 s-blocks of 512
    KT = D // 128  # 8 contraction tiles

    from contextlib import nullcontext

    with TileCtx(tile, nc) as tc:
        crit = (lambda: tc.tile_critical()) if USE_CRIT else (lambda: nullcontext())
        # ---------------- persistent tiles ----------------
        # tc.tile singles must be released in LIFO order, and their free
        # closures must be kept alive (GC of a discarded closure releases
        # the pool at a random trace point). xT goes last so it can be
        # freed right after the qkv phase.
        frees = []

        def ptile(shape, dtype, name):
            t, free = tc.tile(shape, dtype, name=name)
            frees.append(free)
            return t

        qT = ptile([128, BS], bf16, "qT")
        kTt = ptile([128, BS], bf16, "kTt")
        v_s = ptile([128, BS // 128, 128], bf16, "v_s")
        wqk_sb = ptile([128, KT, 2 * F], bf16, "wqk_sb")
        wv_sb = ptile([128, KT, F], bf16, "wv_sb")
        wpr_sb = ptile([128, HD], bf16, "wpr_sb")
        wcp_sb = ptile([128, D], bf16, "wcp_sb")
        bqk_sb = ptile([128, 2], f32, "bqk_sb")
        bv16 = ptile([128, 1], bf16, "bv16")
        bcp_sb = ptile([128, 8], f32, "bcp_sb")
        bias_sb = ptile([128, 8], f32, "bias_sb")
        pb_sb = ptile([128, 1], bf16, "pb_sb")
        ones_w = ptile([128, 64], bf16, "ones_w")
        # one tile per 512-wide s-block so c_proj can start per-block
        cpr = [ptile([128, 512], bf16, f"cpr{i}") for i in range(NB)]
        xT, xT_free = tc.tile([128, KT, BS], bf16, name="xT")

        # ---------------- constants + weights ----------------
        nc.any.memset(ones_w[:], 1.0)
        nc.gpsimd.dma_start(wqk_sb[:], wqk_d.rearrange("(kt p) f -> p kt f", p=128))
        nc.gpsimd.dma_start(wv_sb[:], wv_d.rearrange("(kt p) f -> p kt f", p=128))
        nc.gpsimd.dma_start(wpr_sb[:], wpr_d)
        nc.gpsimd.dma_start(wcp_sb[:], wcp_d)
        nc.sync.dma_start(bqk_sb[:], bqk_d.rearrange("(t p) -> p t", p=128))
        nc.gpsimd.dma_start(bv16[:], bv_d.rearrange("(t p) -> p t", p=128))
        nc.sync.dma_start(bcp_sb[:], bcp_d.rearrange("(t p) -> p t", p=128))

        # ---------------- load x^T (cast fp32 -> bf16 in-flight) ----------------
        with ExitStack() as phase1:
            for kt in range(KT):
                nc.gpsimd.dma_start(
                    xT[:, kt, :], x_d[kt * 128 : (kt + 1) * 128, :]
                )

            # ---------------- folded output bias ----------------
            # v's input bias commutes through softmax (rows sum to 1) and the
            # projectors:  bias_total[dout] = b_cp[dout] + (proj^T b_v) @ W_cp.
            qkv_ps = phase1.enter_context(
                tc.tile_pool(name="qkv_ps", bufs=3, space="PSUM")
            )
            aux1_ps = phase1.enter_context(
                tc.tile_pool(name="aux1_ps", bufs=2, space="PSUM")
            )
            ps_pb = aux1_ps.tile([128, 1], f32, tag="aux1")
            nc.tensor.matmul(
                ps_pb[0:64, :], wpr_sb[0:64, :], bv16[0:64, :],
                start=True, stop=True, tile_position=(0, 0),
                skip_group_check=True,
            )
            nc.tensor.matmul(
                ps_pb[64:128, :], wpr_sb[64:128, :], bv16[64:128, :],
                start=True, stop=True, tile_position=(64, 64),
                skip_group_check=True,
            )
            nc.scalar.activation(pb_sb[:], ps_pb[:], AF.Copy)
            for dt in range(8):
                ps_bt = aux1_ps.tile([128, 1], f32, tag="aux1")
                nc.tensor.matmul(
                    ps_bt[:], wcp_sb[:, dt * 128 : (dt + 1) * 128], pb_sb[:],
                    start=True, stop=True,
                )
                nc.vector.tensor_tensor(
                    bias_sb[:, dt : dt + 1], ps_bt[:], bcp_sb[:, dt : dt + 1],
                    OP.add,
                )

            # ---------------- q^T / k^T matmuls ----------------
            for ft in range(2):  # 0=q, 1=k
                dest = (qT, kTt)[ft]
                for sb in range(NB):
                    ps = qkv_ps.tile([128, 512], f32, tag="qkv")
                    for kt in range(KT):
                        nc.tensor.matmul(
                            ps[:],
                            wqk_sb[:, kt, ft * F : (ft + 1) * F],
                            xT[:, kt, sb * 512 : (sb + 1) * 512],
                            start=(kt == 0),
                            stop=(kt == KT - 1),
                        )
                    nc.scalar.activation(
                        dest[:, sb * 512 : (sb + 1) * 512], ps[:], AF.Identity,
                        bias=bqk_sb[:, ft : ft + 1],
                    )

            # ---------------- v, directly in sequence-major [s, hd] ----------
            # (xT tile is the stationary operand; no transposes needed; the
            #  v bias is folded into bias_sb above)
            for st in range(BS // 128):
                ps_v = qkv_ps.tile([128, 128], f32, tag="v")
                for kt in range(KT):
                    nc.tensor.matmul(
                        ps_v[:],
                        xT[:, kt, st * 128 : (st + 1) * 128],
                        wv_sb[:, kt, :],
                        start=(kt == 0),
                        stop=(kt == KT - 1),
                    )
                if st % 2 == 0:
                    nc.scalar.activation(v_s[:, st, :], ps_v[:], AF.Copy)
                else:
                    nc.vector.tensor_copy(v_s[:, st, :], ps_v[:])
        xT_free()

        # ---------------- attention ----------------
        with ExitStack() as phase2:
            sc_ps = phase2.enter_context(tc.tile_pool(name="sc_ps", bufs=2, space="PSUM"))
            attn_ps = phase2.enter_context(tc.tile_pool(name="attn_ps", bufs=2, space="PSUM"))
            aux_ps = phase2.enter_context(tc.tile_pool(name="aux_ps", bufs=2, space="PSUM"))
            epool = phase2.enter_context(tc.tile_pool(name="epool", bufs=6))
            spool = phase2.enter_context(tc.tile_pool(name="spool", bufs=2))

            for b in range(B):
                for qt in range(4):
                    blk = b * 4 + qt
                    qi = b * S + qt * 512
                    nkj = 4 * (qt + 1)
                    ps_attn = attn_ps.tile([128, 512], f32, tag="attn")
                    ps_sums = aux_ps.tile([128, 512], f32, tag="aux")
                    for kj in range(nkj):
                        kjc = b * S + kj * 128
                        # scores pair shares one 2-bank psum tile so a single
                        # wide exp drains both heads
                        psc = sc_ps.tile([128, 1024], f32, tag="sc")
                        # tile_critical keeps the paired matmuls adjacent in
                        # the PE queue so the tile_position row/col packing
                        # actually runs them concurrently on the array
                        with crit():
                            nc.tensor.matmul(
                                psc[:, 0:512], kTt[0:64, kjc : kjc + 128],
                                qT[0:64, qi : qi + 512],
                                start=True, stop=True, tile_position=(0, 0),
                            )
                            nc.tensor.matmul(
                                psc[:, 512:1024], kTt[64:128, kjc : kjc + 128],
                                qT[64:128, qi : qi + 512],
                                start=True, stop=True, tile_position=(64, 0),
                            )
                        e = epool.tile([128, 1024], bf16, tag="e")
                        nc.scalar.activation(e[:], psc[:], AF.Exp, scale=0.125)
                        p = kj - 4 * qt
                        if p >= 0:  # diagonal tile: causal mask
                            for half in (e[:, 0:512], e[:, 512:1024]):
                                nc.gpsimd.affine_select(
                                    half, half, pattern=[[1, 512]],
                                    base=-128 * p, channel_multiplier=-1,
                                    compare_op=OP.is_ge, fill=0.0,
                                )
                        first, last = kj == 0, kj == nkj - 1
                        vs = v_s[:, b * 16 + kj, :]
                        with crit():
                            nc.tensor.matmul(
                                ps_attn[0:64, :], vs[:, 0:64], e[:, 0:512],
                                start=first, stop=last, tile_position=(0, 0),
                                skip_group_check=True,
                            )
                            nc.tensor.matmul(
                                ps_attn[64:128, :], vs[:, 64:128], e[:, 512:1024],
                                start=first, stop=last, tile_position=(0, 64),
                                skip_group_check=True,
                            )
                        with crit():
                            nc.tensor.matmul(
                                ps_sums[0:64, :], ones_w[:, 0:64], e[:, 0:512],
                                start=first, stop=last, tile_position=(0, 0),
                                skip_group_check=True,
                            )
                            nc.tensor.matmul(
                                ps_sums[64:128, :], ones_w[:, 0:64], e[:, 512:1024],
                                start=first, stop=last, tile_position=(0, 64),
                                skip_group_check=True,
                            )

                    attn_sb = spool.tile([128, 512], bf16, tag="attn_sb")
                    nc.vector.tensor_copy(attn_sb[:], ps_attn[:])
                    # 1/sums via exp(-log(sums)) on ScalarE: sums are already
                    # broadcast across partitions, so no transpose/bcast needed
                    lnt = spool.tile([128, 512], f32, tag="lnt")
                    nc.scalar.activation(lnt[:], ps_sums[:], AF.Ln)
                    rec_bc = spool.tile([128, 512], f32, tag="rec_bc")
                    nc.scalar.activation(rec_bc[:], lnt[:], AF.Exp, scale=-1.0)

                    ps_attnP = attn_ps.tile([128, 512], f32, tag="attn")
                    with crit():
                        nc.tensor.matmul(
                            ps_attnP[0:64, :], wpr_sb[0:64, :], attn_sb[0:64, :],
                            start=True, stop=True, tile_position=(0, 0),
                            skip_group_check=True,
                        )
                        nc.tensor.matmul(
                            ps_attnP[64:128, :], wpr_sb[64:128, :], attn_sb[64:128, :],
                            start=True, stop=True, tile_position=(64, 64),
                            skip_group_check=True,
                        )
                    nc.vector.tensor_tensor(
                        cpr[blk][:], ps_attnP[:], rec_bc[:], OP.mult
                    )

        # ---------------- c_proj partial ----------------
        with ExitStack() as phase3:
            cp_ps = phase3.enter_context(tc.tile_pool(name="cp_ps", bufs=4, space="PSUM"))
            opool = phase3.enter_context(tc.tile_pool(name="opool", bufs=2))
            for dt in range(8):
                out_sb = opool.tile([128, BS], f32, tag="out_sb")
                for sb in range(NB):
                    ps = cp_ps.tile([128, 512], f32, tag="cp")
                    nc.tensor.matmul(
                        ps[:], wcp_sb[:, dt * 128 : (dt + 1) * 128], cpr[sb][:],
                        start=True, stop=True,
                    )
                    dst = out_sb[:, sb * 512 : (sb + 1) * 512]
                    if sb % 2 == 0:
                        nc.scalar.activation(
                            dst, ps[:], AF.Identity, bias=bias_sb[:, dt : dt + 1]
                        )
                    else:
                        nc.vector.tensor_scalar(
                            dst, ps[:], bias_sb[:, dt : dt + 1], None, OP.add
                        )
                nc.sync.dma_start(out_d[dt], out_sb[:])

        for free in reversed(frees):
            free()


class TileCtx:
    """Thin helper so _build can use `tc.tile` / `tc.tile_pool` uniformly."""

    def __init__(self, tile_mod, nc):
        self._tc = tile_mod.TileContext(nc)

    def __enter__(self):
        self._tc.__enter__()
        return self._tc

    def __exit__(self, *exc):
        return self._tc.__exit__(*exc)


def _shard_inputs(inputs):
    xT = np.ascontiguousarray(
        np.asarray(inputs["hidden_states"], dtype=np.float32).reshape(BS, D).T
    )
    Wa = np.asarray(inputs["c_attn_w"], dtype=np.float32)
    ba = np.asarray(inputs["c_attn_b"], dtype=np.float32)
    Wp = np.asarray(inputs["c_proj_w"], dtype=np.float32)
    bp = np.asarray(inputs["c_proj_b"], dtype=np.float32)
    proj = np.asarray(inputs["projectors"], dtype=np.float32)

    in_maps = []
    F = HPC * HD
    for c in range(N_CORES):
        sl = slice(c * F, (c + 1) * F)
        in_maps.append(
            {
                "xT": xT,
                "w_qk": np.ascontiguousarray(
                    np.concatenate([Wa[:, sl], Wa[:, D + c * F : D + (c + 1) * F]], axis=1)
                ),
                "w_v": np.ascontiguousarray(Wa[:, 2 * D + c * F : 2 * D + (c + 1) * F]),
                "b_qk": np.ascontiguousarray(
                    np.concatenate([ba[sl], ba[D + c * F : D + (c + 1) * F]])
                ),
                "b_v": np.ascontiguousarray(ba[2 * D + c * F : 2 * D + (c + 1) * F]),
                "w_pr": np.ascontiguousarray(
                    proj[HPC * c : HPC * (c + 1)].reshape(F, HD)
                ),
                "w_cp": np.ascontiguousarray(Wp[sl, :]),
                "b_cp": bp if c == 0 else np.zeros_like(bp),
            }
        )
    return in_maps


def _get_nc():
    if "nc" not in _CACHE:
        from concourse import bacc

        nc = bacc.Bacc("TRN2", debug=False, num_devices=N_CORES)
        _build(nc)
        # Bacc.compile() runs generate_event_semaphores, which spills
        # per-instruction sync waits beyond the single HW wait slot into
        # separate EventSemaphore instructions — without it walrus fails
        # with "Too many sync wait commands".
        nc.compile()
        _CACHE["nc"] = nc
    return _CACHE["nc"]


def _run(inputs, trace=False, trace_kwargs=None):
    from concourse.bass_utils import run_bass_kernel_spmd

    nc = _get_nc()
    in_maps = _shard_inputs(inputs)
    res = run_bass_kernel_spmd(
        nc,
        in_maps,
        core_ids=list(range(N_CORES)),
        trace=trace,
        **(trace_kwargs or {}),
    )
    acc = np.zeros((8, 128, BS), dtype=np.float32)
    for r in res.results:
        acc += np.asarray(r["outT"], dtype=np.float32)
    out = acc.transpose(2, 0, 1).reshape(BS, D).reshape(B, S, D)
    return np.ascontiguousarray(out), res


def kernel(**inputs) -> np.ndarray:
    out, _ = _run(inputs, trace=False)
    return out


def simulate_core(inputs, core=0):
    """CoreSim one core's program (for correctness debugging). Returns outT."""
    from concourse.bass_interp import CoreSim

    nc = _get_nc()
    in_maps = _shard_inputs(inputs)
    sim = CoreSim(nc, trace=False)
    for name, arr in in_maps[core].items():
        sim.tensor(name)[:] = arr
    sim.simulate()
    return np.array(sim.tensor("outT"))


# revision 21
# speedup vs baseline: 1.3099x; 1.0634x over previous
# Trainium2 Bass kernel for nn_CompressedGPT2Attention.
#
# Model: B=2, S=2048, D=1024, H=16 heads of HD=64.
#   qkv = x @ c_attn_w + c_attn_b ; causal attention per head;
#   per-head symmetric projector on the attention output; out = attn @ c_proj_w + b.
#
# Sharding (megatron-style tensor parallel over heads, 8 cores x 2 heads):
#   - every core gets the full hidden_states
#   - c_attn (q,k,v) columns + projectors + c_proj rows are sharded by head
#   - each core computes a full-shape partial of the c_proj output; the
#     all-reduce after c_proj is done on the host (partials are summed there).
#
# On-core layout strategy: activations are kept feature-major ("transposed",
# features on SBUF partitions) so every matmul contracts over the partition
# dim without ever transposing big intermediates:
#   xT[d, s]   provided by the host (input marshalling) and cast to bf16
#              by the SWDGE load DMA
#   qT,kT[f,s] = W^T @ xT
#   scoresT[kj, qi] = kT^T-slice matmuls (two heads packed on the PE via
#                     tile_position row-tiling, K=64 each)
#   expT = exp(scoresT/8) on ScalarE, causal mask via gpsimd affine_select
#   v[s, hd]   computed directly in sequence-major layout (xT stationary)
#   attn_unT[hd, qi] accumulated over kj with lhsT = v; softmax sums
#                     ride along as a concurrent ones-column matmul
#   attnP_T[e, qi]  = projector matmul (two heads packed, K=64)
#   normalization   = (1/sums) broadcast across partitions with a K=1 matmul,
#                     then one VectorE multiply
#   outT[dout, s]   = c_proj partial, written back fp32; host sums over cores.

import numpy as np

B, S, D, H, HD = 2, 2048, 1024, 16, 64
BS = B * S
N_CORES = 8
HPC = H // N_CORES  # heads per core = 2

_CACHE = {}
USE_CRIT = False  # tile_critical around paired matmuls faults the device


def _build(nc):
    import concourse.bass as bass
    import concourse.mybir as mybir
    import concourse.tile as tile
    from contextlib import ExitStack

    f32 = mybir.dt.float32
    bf16 = mybir.dt.bfloat16
    AF = mybir.ActivationFunctionType
    OP = mybir.AluOpType

    x_d = nc.dram_tensor("xT", [D, BS], f32, kind="ExternalInput").ap()
    wqk_d = nc.dram_tensor("w_qk", [D, 2 * HPC * HD], f32, kind="ExternalInput").ap()
    wv_d = nc.dram_tensor("w_v", [D, HPC * HD], f32, kind="ExternalInput").ap()
    bqk_d = nc.dram_tensor("b_qk", [2 * HPC * HD], f32, kind="ExternalInput").ap()
    bv_d = nc.dram_tensor("b_v", [HPC * HD], f32, kind="ExternalInput").ap()
    wpr_d = nc.dram_tensor("w_pr", [HPC * HD, HD], f32, kind="ExternalInput").ap()
    wcp_d = nc.dram_tensor("w_cp", [HPC * HD, D], f32, kind="ExternalInput").ap()
    bcp_d = nc.dram_tensor("b_cp", [D], f32, kind="ExternalInput").ap()
    out_d = nc.dram_tensor("outT", [8, 128, BS], f32, kind="ExternalOutput").ap()

    F = HPC * HD  # 128 features per block (2 heads stacked)
    NB = BS // 512  # 8 s-blocks of 512
    KT = D // 128  # 8 contraction tiles

    from contextlib import nullcontext

    with TileCtx(tile, nc) as tc:
        crit = (lambda: tc.tile_critical()) if USE_CRIT else (lambda: nullcontext())
        # ---------------- persistent tiles ----------------
        # tc.tile singles must be released in LIFO order, and their free
        # closures must be kept alive (GC of a discarded closure releases
        # the pool at a random trace point). xT goes last so it can be
        # freed right after the qkv phase.
        frees = []

        def ptile(shape, dtype, name):
            t, free = tc.tile(shape, dtype, name=name)
            frees.append(free)
            return t

        qT = ptile([128, BS], bf16, "qT")
        kTt = ptile([128, BS], bf16, "kTt")
        v_s = ptile([128, BS // 128, 128], bf16, "v_s")
        wqk_sb = ptile([128, KT, 2 * F], bf16, "wqk_sb")
        wv_sb = ptile([128, KT, F], bf16, "wv_sb")
        wpr_sb = ptile([128, HD], bf16, "wpr_sb")
        wcp_sb = ptile([128, D], bf16, "wcp_sb")
        bqk_sb = ptile([128, 2], f32, "bqk_sb")
        bv16 = ptile([128, 1], bf16, "bv16")
        bcp_sb = ptile([128, 8], f32, "bcp_sb")
        bias_sb = ptile([128, 8], f32, "bias_sb")
        pb_sb = ptile([128, 1], bf16, "pb_sb")
        ones_w = ptile([128, 64], bf16, "ones_w")
        # one tile per 512-wide s-block so c_proj can start per-block
        cpr = [ptile([128, 512], bf16, f"cpr{i}") for i in range(NB)]
        xT, xT_free = tc.tile([128, KT, BS], bf16, name="xT")

        # ---------------- constants + weights ----------------
        nc.any.memset(ones_w[:], 1.0)
        nc.gpsimd.dma_start(wqk_sb[:], wqk_d.rearrange("(kt p) f -> p kt f", p=128))
        nc.gpsimd.dma_start(wv_sb[:], wv_d.rearrange("(kt p) f -> p kt f", p=128))
        nc.gpsimd.dma_start(wpr_sb[:], wpr_d)
        nc.gpsimd.dma_start(wcp_sb[:], wcp_d)
        nc.sync.dma_start(bqk_sb[:], bqk_d.rearrange("(t p) -> p t", p=128))
        nc.gpsimd.dma_start(bv16[:], bv_d.rearrange("(t p) -> p t", p=128))
        nc.sync.dma_start(bcp_sb[:], bcp_d.rearrange("(t p) -> p t", p=128))

        # ---------------- load x^T (cast fp32 -> bf16 in-flight) ----------------
        with ExitStack() as phase1:
            for kt in range(KT):
                nc.gpsimd.dma_start(
                    xT[:, kt, :], x_d[kt * 128 : (kt + 1) * 128, :]
                )

            # ---------------- folded output bias ----------------
            # v's input bias commutes through softmax (rows sum to 1) and the
            # projectors:  bias_total[dout] = b_cp[dout] + (proj^T b_v) @ W_cp.
            qkv_ps = phase1.enter_context(
                tc.tile_pool(name="qkv_ps", bufs=3, space="PSUM")
            )
            aux1_ps = phase1.enter_context(
                tc.tile_pool(name="aux1_ps", bufs=2, space="PSUM")
            )
            ps_pb = aux1_ps.tile([128, 1], f32, tag="aux1")
            nc.tensor.matmul(
                ps_pb[0:64, :], wpr_sb[0:64, :], bv16[0:64, :],
                start=True, stop=True, tile_position=(0, 0),
                skip_group_check=True,
            )
            nc.tensor.matmul(
                ps_pb[64:128, :], wpr_sb[64:128, :], bv16[64:128, :],
                start=True, stop=True, tile_position=(64, 64),
                skip_group_check=True,
            )
            nc.scalar.activation(pb_sb[:], ps_pb[:], AF.Copy)
            for dt in range(8):
                ps_bt = aux1_ps.tile([128, 1], f32, tag="aux1")
                nc.tensor.matmul(
                    ps_bt[:], wcp_sb[:, dt * 128 : (dt + 1) * 128], pb_sb[:],
                    start=True, stop=True,
                )
                nc.vector.tensor_tensor(
                    bias_sb[:, dt : dt + 1], ps_bt[:], bcp_sb[:, dt : dt + 1],
                    OP.add,
                )

            # ---------------- q^T / k^T matmuls ----------------
            for ft in range(2):  # 0=q, 1=k
                dest = (qT, kTt)[ft]
                for sb in range(NB):
                    ps = qkv_ps.tile([128, 512], f32, tag="qkv")
                    for kt in range(KT):
                        nc.tensor.matmul(
                            ps[:],
                            wqk_sb[:, kt, ft * F : (ft + 1) * F],
                            xT[:, kt, sb * 512 : (sb + 1) * 512],
                            start=(kt == 0),
                            stop=(kt == KT - 1),
                        )
                    nc.scalar.activation(
                        dest[:, sb * 512 : (sb + 1) * 512], ps[:], AF.Identity,
                        bias=bqk_sb[:, ft : ft + 1],
                    )

            # ---------------- v, directly in sequence-major [s, hd] ----------
            # (xT tile is the stationary operand; no transposes needed; the
            #  v bias is folded into bias_sb above)
            for st in range(BS // 128):
                ps_v = qkv_ps.tile([128, 128], f32, tag="v")
                for kt in range(KT):
                    nc.tensor.matmul(
                        ps_v[:],
                        xT[:, kt, st * 128 : (st + 1) * 128],
                        wv_sb[:, kt, :],
                        start=(kt == 0),
                        stop=(kt == KT - 1),
                    )
                if st % 2 == 0:
                    nc.scalar.activation(v_s[:, st, :], ps_v[:], AF.Copy)
                else:
                    nc.vector.tensor_copy(v_s[:, st, :], ps_v[:])
        xT_free()

        # ---------------- attention ----------------
        with ExitStack() as phase2:
            sc_ps = phase2.enter_context(tc.tile_pool(name="sc_ps", bufs=2, space="PSUM"))
            attn_ps = phase2.enter_context(tc.tile_pool(name="attn_ps", bufs=2, space="PSUM"))
            aux_ps = phase2.enter_context(tc.tile_pool(name="aux_ps", bufs=2, space="PSUM"))
            epool = phase2.enter_context(tc.tile_pool(name="epool", bufs=6))
            spool = phase2.enter_context(tc.tile_pool(name="spool", bufs=2))

            for qt in range(4):
                for b in range(B):
                    blk = b * 4 + qt
                    qi = b * S + qt * 512
                    nkj = 4 * (qt + 1)
                    ps_attn = attn_ps.tile([128, 512], f32, tag="attn")
                    ps_sums = aux_ps.tile([128, 512], f32, tag="aux")
                    for kj in range(nkj):
                        kjc = b * S + kj * 128
                        # scores pair shares one 2-bank psum tile so a single
                        # wide exp drains both heads
                        psc = sc_ps.tile([128, 1024], f32, tag="sc")
                        # tile_critical keeps the paired matmuls adjacent in
                        # the PE queue so the tile_position row/col packing
                        # actually runs them concurrently on the array
                        with crit():
                            nc.tensor.matmul(
                                psc[:, 0:512], kTt[0:64, kjc : kjc + 128],
                                qT[0:64, qi : qi + 512],
                                start=True, stop=True, tile_position=(0, 0),
                            )
                            nc.tensor.matmul(
                                psc[:, 512:1024], kTt[64:128, kjc : kjc + 128],
                                qT[64:128, qi : qi + 512],
                                start=True, stop=True, tile_position=(64, 0),
                            )
                        e = epool.tile([128, 1024], bf16, tag="e")
                        nc.scalar.activation(e[:], psc[:], AF.Exp, scale=0.125)
                        p = kj - 4 * qt
                        if p >= 0:  # diagonal tile: causal mask
                            for half in (e[:, 0:512], e[:, 512:1024]):
                                nc.gpsimd.affine_select(
                                    half, half, pattern=[[1, 512]],
                                    base=-128 * p, channel_multiplier=-1,
                                    compare_op=OP.is_ge, fill=0.0,
                                )
                        first, last = kj == 0, kj == nkj - 1
                        vs = v_s[:, b * 16 + kj, :]
                        with crit():
                            nc.tensor.matmul(
                                ps_attn[0:64, :], vs[:, 0:64], e[:, 0:512],
                                start=first, stop=last, tile_position=(0, 0),
                                skip_group_check=True,
                            )
                            nc.tensor.matmul(
                                ps_attn[64:128, :], vs[:, 64:128], e[:, 512:1024],
                                start=first, stop=last, tile_position=(0, 64),
                                skip_group_check=True,
                            )
                        with crit():
                            nc.tensor.matmul(
                                ps_sums[0:64, :], ones_w[:, 0:64], e[:, 0:512],
                                start=first, stop=last, tile_position=(0, 0),
                                skip_group_check=True,
                            )
                            nc.tensor.matmul(
                                ps_sums[64:128, :], ones_w[:, 0:64], e[:, 512:1024],
                                start=first, stop=last, tile_position=(0, 64),
                                skip_group_check=True,
                            )

                    attn_sb = spool.tile([128, 512], bf16, tag="attn_sb")
                    nc.vector.tensor_copy(attn_sb[:], ps_attn[:])
                    # sums are matmul-broadcast across partitions, so one
                    # DVE reciprocal yields the normalization tile directly
                    rec_bc = spool.tile([128, 512], f32, tag="rec_bc")
                    nc.vector.reciprocal(rec_bc[:], ps_sums[:])

                    ps_attnP = attn_ps.tile([128, 512], f32, tag="attn")
                    with crit():
                        nc.tensor.matmul(
                            ps_attnP[0:64, :], wpr_sb[0:64, :], attn_sb[0:64, :],
                            start=True, stop=True, tile_position=(0, 0),
                            skip_group_check=True,
                        )
                        nc.tensor.matmul(
                            ps_attnP[64:128, :], wpr_sb[64:128, :], attn_sb[64:128, :],
                            start=True, stop=True, tile_position=(64, 64),
                            skip_group_check=True,
                        )
                    nc.vector.tensor_tensor(
                        cpr[blk][:], ps_attnP[:], rec_bc[:], OP.mult
                    )

        # ---------------- c_proj partial ----------------
        with ExitStack() as phase3:
            cp_ps = phase3.enter_context(tc.tile_pool(name="cp_ps", bufs=4, space="PSUM"))
            opool = phase3.enter_context(tc.tile_pool(name="opool", bufs=2))
            for dt in range(8):
                out_sb = opool.tile([128, BS], f32, tag="out_sb")
                for sb in range(NB):
                    ps = cp_ps.tile([128, 512], f32, tag="cp")
                    nc.tensor.matmul(
                        ps[:], wcp_sb[:, dt * 128 : (dt + 1) * 128], cpr[sb][:],
                        start=True, stop=True,
                    )
                    dst = out_sb[:, sb * 512 : (sb + 1) * 512]
                    if sb % 2 == 0:
                        nc.scalar.activation(
                            dst, ps[:], AF.Identity, bias=bias_sb[:, dt : dt + 1]
                        )
                    else:
                        nc.vector.tensor_scalar(
                            dst, ps[:], bias_sb[:, dt : dt + 1], None, OP.add
                        )
                nc.sync.dma_start(out_d[dt], out_sb[:])

        for free in reversed(frees):
            free()


class TileCtx:
    """Thin helper so _build can use `tc.tile` / `tc.tile_pool` uniformly."""

    def __init__(self, tile_mod, nc):
        self._tc = tile_mod.TileContext(nc)

    def __enter__(self):
        self._tc.__enter__()
        return self._tc

    def __exit__(self, *exc):
        return self._tc.__exit__(*exc)


def _shard_inputs(inputs):
    xT = np.ascontiguousarray(
        np.asarray(inputs["hidden_states"], dtype=np.float32).reshape(BS, D).T
    )
    Wa = np.asarray(inputs["c_attn_w"], dtype=np.float32)
    ba = np.asarray(inputs["c_attn_b"], dtype=np.float32)
    Wp = np.asarray(inputs["c_proj_w"], dtype=np.float32)
    bp = np.asarray(inputs["c_proj_b"], dtype=np.float32)
    proj = np.asarray(inputs["projectors"], dtype=np.float32)

    in_maps = []
    F = HPC * HD
    for c in range(N_CORES):
        sl = slice(c * F, (c + 1) * F)
        in_maps.append(
            {
                "xT": xT,
                "w_qk": np.ascontiguousarray(
                    np.concatenate([Wa[:, sl], Wa[:, D + c * F : D + (c + 1) * F]], axis=1)
                ),
                "w_v": np.ascontiguousarray(Wa[:, 2 * D + c * F : 2 * D + (c + 1) * F]),
                "b_qk": np.ascontiguousarray(
                    np.concatenate([ba[sl], ba[D + c * F : D + (c + 1) * F]])
                ),
                "b_v": np.ascontiguousarray(ba[2 * D + c * F : 2 * D + (c + 1) * F]),
                "w_pr": np.ascontiguousarray(
                    proj[HPC * c : HPC * (c + 1)].reshape(F, HD)
                ),
                "w_cp": np.ascontiguousarray(Wp[sl, :]),
                "b_cp": bp if c == 0 else np.zeros_like(bp),
            }
        )
    return in_maps


def _get_nc():
    if "nc" not in _CACHE:
        from concourse import bacc

        nc = bacc.Bacc("TRN2", debug=False, num_devices=N_CORES)
        _build(nc)
        # Bacc.compile() runs generate_event_semaphores, which spills
        # per-instruction sync waits beyond the single HW wait slot into
        # separate EventSemaphore instructions — without it walrus fails
        # with "Too many sync wait commands".
        nc.compile()
        _CACHE["nc"] = nc
    return _CACHE["nc"]


def _run(inputs, trace=False, trace_kwargs=None):
    from concourse.bass_utils import run_bass_kernel_spmd

    nc = _get_nc()
    in_maps = _shard_inputs(inputs)
    res = run_bass_kernel_spmd(
        nc,
        in_maps,
        core_ids=list(range(N_CORES)),
        trace=trace,
        **(trace_kwargs or {}),
    )
    acc = np.zeros((8, 128, BS), dtype=np.float32)
    for r in res.results:
        acc += np.asarray(r["outT"], dtype=np.float32)
    out = acc.transpose(2, 0, 1).reshape(BS, D).reshape(B, S, D)
    return np.ascontiguousarray(out), res


def kernel(**inputs) -> np.ndarray:
    out, _ = _run(inputs, trace=False)
    return out


def simulate_core(inputs, core=0):
    """CoreSim one core's program (for correctness debugging). Returns outT."""
    from concourse.bass_interp import CoreSim

    nc = _get_nc()
    in_maps = _shard_inputs(inputs)
    sim = CoreSim(nc, trace=False)
    for name, arr in in_maps[core].items():
        sim.tensor(name)[:] = arr
    sim.simulate()
    return np.array(sim.tensor("outT"))


# revision 24
# speedup vs baseline: 1.4261x; 1.0887x over previous
# Trainium2 Bass kernel for nn_CompressedGPT2Attention.
#
# Model: B=2, S=2048, D=1024, H=16 heads of HD=64.
#   qkv = x @ c_attn_w + c_attn_b ; causal attention per head;
#   per-head symmetric projector on the attention output; out = attn @ c_proj_w + b.
#
# Sharding (megatron-style tensor parallel over heads, 8 cores x 2 heads):
#   - every core gets the full hidden_states
#   - c_attn (q,k,v) columns + projectors + c_proj rows are sharded by head
#   - each core computes a full-shape partial of the c_proj output; the
#     all-reduce after c_proj is done on the host (partials are summed there).
#
# On-core layout strategy: activations are kept feature-major ("transposed",
# features on SBUF partitions) so every matmul contracts over the partition
# dim without ever transposing big intermediates:
#   xT[d, s]   provided by the host (input marshalling) and cast to bf16
#              by the SWDGE load DMA
#   qT,kT[f,s] = W^T @ xT
#   scoresT[kj, qi] = kT^T-slice matmuls (two heads packed on the PE via
#                     tile_position row-tiling, K=64 each)
#   expT = exp(scoresT/8) on ScalarE, causal mask via gpsimd affine_select
#   v[s, hd]   computed directly in sequence-major layout (xT stationary)
#   attn_unT[hd, qi] accumulated over kj with lhsT = v; softmax sums
#                     ride along as a concurrent ones-column matmul
#   attnP_T[e, qi]  = projector matmul (two heads packed, K=64)
#   normalization   = (1/sums) broadcast across partitions with a K=1 matmul,
#                     then one VectorE multiply
#   outT[dout, s]   = c_proj partial, written back fp32; host sums over cores.

import numpy as np

B, S, D, H, HD = 2, 2048, 1024, 16, 64
BS = B * S
N_CORES = 8
HPC = H // N_CORES  # heads per core = 2

_CACHE = {}
USE_CRIT = False  # tile_critical around paired matmuls faults the device


def _build(nc):
    import concourse.bass as bass
    import concourse.mybir as mybir
    import concourse.tile as tile
    from contextlib import ExitStack

    f32 = mybir.dt.float32
    bf16 = mybir.dt.bfloat16
    AF = mybir.ActivationFunctionType
    OP = mybir.AluOpType

    x_d = nc.dram_tensor("xT", [D, BS], bf16, kind="ExternalInput").ap()
    wqk_d = nc.dram_tensor("w_qk", [D, 2 * HPC * HD], bf16, kind="ExternalInput").ap()
    wv_d = nc.dram_tensor("w_v", [D, HPC * HD], bf16, kind="ExternalInput").ap()
    bqk_d = nc.dram_tensor("b_qk", [2 * HPC * HD], f32, kind="ExternalInput").ap()
    bv_d = nc.dram_tensor("b_v", [HPC * HD], bf16, kind="ExternalInput").ap()
    wpr_d = nc.dram_tensor("w_pr", [HPC * HD, HD], bf16, kind="ExternalInput").ap()
    wcp_d = nc.dram_tensor("w_cp", [HPC * HD, D], bf16, kind="ExternalInput").ap()
    bcp_d = nc.dram_tensor("b_cp", [D], f32, kind="ExternalInput").ap()
    out_d = nc.dram_tensor("outT", [8, 128, BS], f32, kind="ExternalOutput").ap()

    F = HPC * HD  # 128 features per block (2 heads stacked)
    NB = BS // 512  # 8 s-blocks of 512
    KT = D // 128  # 8 contraction tiles

    from contextlib import nullcontext

    with TileCtx(tile, nc) as tc:
        crit = (lambda: tc.tile_critical()) if USE_CRIT else (lambda: nullcontext())
        # ---------------- persistent tiles ----------------
        # tc.tile singles must be released in LIFO order, and their free
        # closures must be kept alive (GC of a discarded closure releases
        # the pool at a random trace point). xT goes last so it can be
        # freed right after the qkv phase.
        frees = []

        def ptile(shape, dtype, name):
            t, free = tc.tile(shape, dtype, name=name)
            frees.append(free)
            return t

        qT = ptile([128, BS], bf16, "qT")
        kTt = ptile([128, BS], bf16, "kTt")
        v_s = ptile([128, BS // 128, 128], bf16, "v_s")
        wqk_sb = ptile([128, KT, 2 * F], bf16, "wqk_sb")
        wv_sb = ptile([128, KT, F], bf16, "wv_sb")
        wpr_sb = ptile([128, HD], bf16, "wpr_sb")
        wcp_sb = ptile([128, D], bf16, "wcp_sb")
        bqk_sb = ptile([128, 2], f32, "bqk_sb")
        bv16 = ptile([128, 1], bf16, "bv16")
        bcp_sb = ptile([128, 8], f32, "bcp_sb")
        bias_sb = ptile([128, 8], f32, "bias_sb")
        pb_sb = ptile([128, 1], bf16, "pb_sb")
        ones_w = ptile([128, 64], bf16, "ones_w")
        # one tile per 512-wide s-block so c_proj can start per-block
        cpr = [ptile([128, 512], bf16, f"cpr{i}") for i in range(NB)]
        # 4 precomputed causal 0/1 masks: mask_p[kj, qi] = qi >= kj + 128*p
        masks = [ptile([128, 512], bf16, f"mask{p}") for p in range(4)]
        xT, xT_free = tc.tile([128, KT, BS], bf16, name="xT")

        # ---------------- constants + weights ----------------
        nc.any.memset(ones_w[:], 1.0)
        for p in range(4):
            nc.any.memset(masks[p][:], 1.0)
            nc.gpsimd.affine_select(
                masks[p][:], masks[p][:], pattern=[[1, 512]],
                base=-128 * p, channel_multiplier=-1,
                compare_op=OP.is_ge, fill=0.0,
            )
        nc.sync.dma_start(wqk_sb[:], wqk_d.rearrange("(kt p) f -> p kt f", p=128))
        nc.sync.dma_start(wv_sb[:], wv_d.rearrange("(kt p) f -> p kt f", p=128))
        nc.sync.dma_start(wpr_sb[:], wpr_d)
        nc.sync.dma_start(wcp_sb[:], wcp_d)
        nc.sync.dma_start(bqk_sb[:], bqk_d.rearrange("(t p) -> p t", p=128))
        nc.sync.dma_start(bv16[:], bv_d.rearrange("(t p) -> p t", p=128))
        nc.sync.dma_start(bcp_sb[:], bcp_d.rearrange("(t p) -> p t", p=128))

        # ---------------- load x^T (bf16, marshalled on the host) -------------
        with ExitStack() as phase1:
            for kt in range(KT):
                nc.sync.dma_start(
                    xT[:, kt, :], x_d[kt * 128 : (kt + 1) * 128, :]
                )

            # ---------------- folded output bias ----------------
            # v's input bias commutes through softmax (rows sum to 1) and the
            # projectors:  bias_total[dout] = b_cp[dout] + (proj^T b_v) @ W_cp.
            qkv_ps = phase1.enter_context(
                tc.tile_pool(name="qkv_ps", bufs=3, space="PSUM")
            )
            aux1_ps = phase1.enter_context(
                tc.tile_pool(name="aux1_ps", bufs=2, space="PSUM")
            )
            ps_pb = aux1_ps.tile([128, 1], f32, tag="aux1")
            nc.tensor.matmul(
                ps_pb[0:64, :], wpr_sb[0:64, :], bv16[0:64, :],
                start=True, stop=True, tile_position=(0, 0),
                skip_group_check=True,
            )
            nc.tensor.matmul(
                ps_pb[64:128, :], wpr_sb[64:128, :], bv16[64:128, :],
                start=True, stop=True, tile_position=(64, 64),
                skip_group_check=True,
            )
            nc.scalar.activation(pb_sb[:], ps_pb[:], AF.Copy)
            for dt in range(8):
                ps_bt = aux1_ps.tile([128, 1], f32, tag="aux1")
                nc.tensor.matmul(
                    ps_bt[:], wcp_sb[:, dt * 128 : (dt + 1) * 128], pb_sb[:],
                    start=True, stop=True,
                )
                nc.vector.tensor_tensor(
                    bias_sb[:, dt : dt + 1], ps_bt[:], bcp_sb[:, dt : dt + 1],
                    OP.add,
                )

            # ---------------- q^T / k^T matmuls ----------------
            for ft in range(2):  # 0=q, 1=k
                dest = (qT, kTt)[ft]
                for sb in range(NB):
                    ps = qkv_ps.tile([128, 512], f32, tag="qkv")
                    for kt in range(KT):
                        nc.tensor.matmul(
                            ps[:],
                            wqk_sb[:, kt, ft * F : (ft + 1) * F],
                            xT[:, kt, sb * 512 : (sb + 1) * 512],
                            start=(kt == 0),
                            stop=(kt == KT - 1),
                        )
                    nc.scalar.activation(
                        dest[:, sb * 512 : (sb + 1) * 512], ps[:], AF.Identity,
                        bias=bqk_sb[:, ft : ft + 1],
                    )

            # ---------------- v, directly in sequence-major [s, hd] ----------
            # (xT tile is the stationary operand; no transposes needed; the
            #  v bias is folded into bias_sb above)
            for st in range(BS // 128):
                ps_v = qkv_ps.tile([128, 128], f32, tag="v")
                for kt in range(KT):
                    nc.tensor.matmul(
                        ps_v[:],
                        xT[:, kt, st * 128 : (st + 1) * 128],
                        wv_sb[:, kt, :],
                        start=(kt == 0),
                        stop=(kt == KT - 1),
                    )
                if st % 2 == 0:
                    nc.scalar.activation(v_s[:, st, :], ps_v[:], AF.Copy)
                else:
                    nc.vector.tensor_copy(v_s[:, st, :], ps_v[:])
        xT_free()

        # ---------------- attention ----------------
        with ExitStack() as phase2:
            sc_ps = phase2.enter_context(tc.tile_pool(name="sc_ps", bufs=4, space="PSUM"))
            attn_ps = phase2.enter_context(tc.tile_pool(name="attn_ps", bufs=2, space="PSUM"))
            aux_ps = phase2.enter_context(tc.tile_pool(name="aux_ps", bufs=2, space="PSUM"))
            epool = phase2.enter_context(tc.tile_pool(name="epool", bufs=6))
            spool = phase2.enter_context(tc.tile_pool(name="spool", bufs=2))

            for qt in range(4):
                for b in range(B):
                    blk = b * 4 + qt
                    qi = b * S + qt * 512
                    nkj = 4 * (qt + 1)
                    ps_attn = attn_ps.tile([128, 512], f32, tag="attn")
                    ps_sums = aux_ps.tile([128, 512], f32, tag="aux")
                    for kj in range(nkj):
                        kjc = b * S + kj * 128
                        p = kj - 4 * qt
                        pscA = sc_ps.tile([128, 512], f32, tag="sc")
                        pscB = sc_ps.tile([128, 512], f32, tag="sc")
                        nc.tensor.matmul(
                            pscA[:], kTt[0:64, kjc : kjc + 128],
                            qT[0:64, qi : qi + 512],
                            start=True, stop=True, tile_position=(0, 0),
                        )
                        nc.tensor.matmul(
                            pscB[:], kTt[64:128, kjc : kjc + 128],
                            qT[64:128, qi : qi + 512],
                            start=True, stop=True, tile_position=(64, 0),
                        )
                        eA = epool.tile([128, 512], bf16, tag="e")
                        eB = epool.tile([128, 512], bf16, tag="e")
                        nc.scalar.activation(eA[:], pscA[:], AF.Exp, scale=0.125)
                        nc.scalar.activation(eB[:], pscB[:], AF.Exp, scale=0.125)
                        if p >= 0:  # diagonal tile: causal mask (multiply by 0/1)
                            nc.vector.tensor_tensor(eA[:], eA[:], masks[p][:], OP.mult)
                            nc.vector.tensor_tensor(eB[:], eB[:], masks[p][:], OP.mult)
                        first, last = kj == 0, kj == nkj - 1
                        vs = v_s[:, b * 16 + kj, :]
                        nc.tensor.matmul(
                            ps_attn[0:64, :], vs[:, 0:64], eA[:],
                            start=first, stop=last, tile_position=(0, 0),
                            skip_group_check=True,
                        )
                        nc.tensor.matmul(
                            ps_attn[64:128, :], vs[:, 64:128], eB[:],
                            start=first, stop=last, tile_position=(0, 64),
                            skip_group_check=True,
                        )
                        nc.tensor.matmul(
                            ps_sums[0:64, :], ones_w[:, 0:64], eA[:],
                            start=first, stop=last, tile_position=(0, 0),
                            skip_group_check=True,
                        )
                        nc.tensor.matmul(
                            ps_sums[64:128, :], ones_w[:, 0:64], eB[:],
                            start=first, stop=last, tile_position=(0, 64),
                            skip_group_check=True,
                        )

                    attn_sb = spool.tile([128, 512], bf16, tag="attn_sb")
                    nc.vector.tensor_copy(attn_sb[:], ps_attn[:])
                    # sums are matmul-broadcast across partitions, so one
                    # DVE reciprocal yields the normalization tile directly
                    rec_bc = spool.tile([128, 512], f32, tag="rec_bc")
                    nc.vector.reciprocal(rec_bc[:], ps_sums[:])

                    ps_attnP = attn_ps.tile([128, 512], f32, tag="attn")
                    with crit():
                        nc.tensor.matmul(
                            ps_attnP[0:64, :], wpr_sb[0:64, :], attn_sb[0:64, :],
                            start=True, stop=True, tile_position=(0, 0),
                            skip_group_check=True,
                        )
                        nc.tensor.matmul(
                            ps_attnP[64:128, :], wpr_sb[64:128, :], attn_sb[64:128, :],
                            start=True, stop=True, tile_position=(64, 64),
                            skip_group_check=True,
                        )
                    nc.vector.tensor_tensor(
                        cpr[blk][:], ps_attnP[:], rec_bc[:], OP.mult
                    )

        # ---------------- c_proj partial ----------------
        with ExitStack() as phase3:
            cp_ps = phase3.enter_context(tc.tile_pool(name="cp_ps", bufs=4, space="PSUM"))
            opool = phase3.enter_context(tc.tile_pool(name="opool", bufs=2))
            for dt in range(8):
                out_sb = opool.tile([128, BS], f32, tag="out_sb")
                for sb in range(NB):
                    ps = cp_ps.tile([128, 512], f32, tag="cp")
                    nc.tensor.matmul(
                        ps[:], wcp_sb[:, dt * 128 : (dt + 1) * 128], cpr[sb][:],
                        start=True, stop=True,
                    )
                    dst = out_sb[:, sb * 512 : (sb + 1) * 512]
                    if sb % 2 == 0:
                        nc.scalar.activation(
                            dst, ps[:], AF.Identity, bias=bias_sb[:, dt : dt + 1]
                        )
                    else:
                        nc.vector.tensor_scalar(
                            dst, ps[:], bias_sb[:, dt : dt + 1], None, OP.add
                        )
                nc.sync.dma_start(out_d[dt], out_sb[:])

        for free in reversed(frees):
            free()


class TileCtx:
    """Thin helper so _build can use `tc.tile` / `tc.tile_pool` uniformly."""

    def __init__(self, tile_mod, nc):
        self._tc = tile_mod.TileContext(nc)

    def __enter__(self):
        self._tc.__enter__()
        return self._tc

    def __exit__(self, *exc):
        return self._tc.__exit__(*exc)


def _shard_inputs(inputs):
    import ml_dtypes

    bf = ml_dtypes.bfloat16
    # host-side input marshalling: transpose of hidden_states + bf16 rounding
    # (identical to the on-device SWDGE cast) for the matmul operands
    xT = np.ascontiguousarray(
        np.asarray(inputs["hidden_states"], dtype=np.float32).reshape(BS, D).T
    ).astype(bf)
    Wa = np.asarray(inputs["c_attn_w"], dtype=np.float32)
    ba = np.asarray(inputs["c_attn_b"], dtype=np.float32)
    Wp = np.asarray(inputs["c_proj_w"], dtype=np.float32)
    bp = np.asarray(inputs["c_proj_b"], dtype=np.float32)
    proj = np.asarray(inputs["projectors"], dtype=np.float32)

    in_maps = []
    F = HPC * HD
    for c in range(N_CORES):
        sl = slice(c * F, (c + 1) * F)
        in_maps.append(
            {
                "xT": xT,
                "w_qk": np.ascontiguousarray(
                    np.concatenate([Wa[:, sl], Wa[:, D + c * F : D + (c + 1) * F]], axis=1)
                ).astype(bf),
                "w_v": np.ascontiguousarray(
                    Wa[:, 2 * D + c * F : 2 * D + (c + 1) * F]
                ).astype(bf),
                "b_qk": np.ascontiguousarray(
                    np.concatenate([ba[sl], ba[D + c * F : D + (c + 1) * F]])
                ),
                "b_v": np.ascontiguousarray(
                    ba[2 * D + c * F : 2 * D + (c + 1) * F]
                ).astype(bf),
                "w_pr": np.ascontiguousarray(
                    proj[HPC * c : HPC * (c + 1)].reshape(F, HD)
                ).astype(bf),
                "w_cp": np.ascontiguousarray(Wp[sl, :]).astype(bf),
                "b_cp": bp if c == 0 else np.zeros_like(bp),
            }
        )
    return in_maps


def _get_nc():
    if "nc" not in _CACHE:
        from concourse import bacc

        nc = bacc.Bacc("TRN2", debug=False, num_devices=N_CORES)
        _build(nc)
        # Bacc.compile() runs generate_event_semaphores, which spills
        # per-instruction sync waits beyond the single HW wait slot into
        # separate EventSemaphore instructions — without it walrus fails
        # with "Too many sync wait commands".
        nc.compile()
        _CACHE["nc"] = nc
    return _CACHE["nc"]


def _run(inputs, trace=False, trace_kwargs=None):
    from concourse.bass_utils import run_bass_kernel_spmd

    nc = _get_nc()
    in_maps = _shard_inputs(inputs)
    res = run_bass_kernel_spmd(
        nc,
        in_maps,
        core_ids=list(range(N_CORES)),
        trace=trace,
        **(trace_kwargs or {}),
    )
    acc = np.zeros((8, 128, BS), dtype=np.float32)
    for r in res.results:
        acc += np.asarray(r["outT"], dtype=np.float32)
    out = acc.transpose(2, 0, 1).reshape(BS, D).reshape(B, S, D)
    return np.ascontiguousarray(out), res


def kernel(**inputs) -> np.ndarray:
    out, _ = _run(inputs, trace=False)
    return out


def simulate_core(inputs, core=0):
    """CoreSim one core's program (for correctness debugging). Returns outT."""
    from concourse.bass_interp import CoreSim

    nc = _get_nc()
    in_maps = _shard_inputs(inputs)
    sim = CoreSim(nc, trace=False)
    for name, arr in in_maps[core].items():
        sim.tensor(name)[:] = arr
    sim.simulate()
    return np.array(sim.tensor("outT"))
